# revision 15
# baseline (speedup 1.0000x reference)
"""BERT-base + 2-layer BiLSTM + CRF negative-log-likelihood on 8 Trainium2 NeuronCores.

Sharding: data-parallel over the batch (B=4). Core c computes batch element
c//2 end-to-end (pairs are redundant — zero cross-core traffic until the final
scalar AllReduce, each core contributes (denom-num)/2 of its batch element).

BERT: token-major fp32 residual stream, feature-major bf16 matmul operands
(produced via DMA transpose), fp32 PSUM accumulation.  Softmax skips the max
subtraction (scores are bounded ~|2|) so attention runs in exp space with one
reciprocal pass per layer.  LSTM: recurrent matmuls accumulate onto the
precomputed input gates held in PSUM; all four gate nonlinearities are a
single batched sigmoid (tanh(g) = 2*sigmoid(2g)-1 with the 2x folded into the
weights on the host); the cell update is one fused custom DVE op.  CRF: the
forward recursion runs in probability space (p' = exp(trans)^T p * exp(em_t))
as tiny 9x9 matmuls with periodic renormalisation; the two sequence halves run
as independent chains (vector scan + transposed matrix product) and are joined
at the end.
"""

import sys, os

for _p in ("/opt/trn_rl_repo", os.path.expanduser("~/.axon_site/_ro/trn_rl_repo")):
    if os.path.isdir(_p) and _p not in sys.path:
        sys.path.append(_p)

import numpy as np
import ml_dtypes

import concourse.bass as bass
import concourse.tile as tile
from concourse import mybir, bacc
from concourse.bass_utils import run_bass_kernel_spmd
from concourse.masks import make_identity

AF = mybir.ActivationFunctionType
OP = mybir.AluOpType
BF16 = mybir.dt.bfloat16
F32 = mybir.dt.float32
I32 = mybir.dt.int32

# model dims
B, S, H, L, NH, DH, FF = 4, 256, 768, 12, 12, 64, 3072
V, P, R, T = 30522, 512, 128, 9
TC = S // 128          # token chunks
HC = H // 128          # hidden chunks
FC = FF // 128         # ff chunks
N_CORES = 8
RENORM = 16            # CRF renormalisation interval

# ---------------------------------------------------------------- custom DVE op
from concourse.dve_spec import Spec, Src0, Src1, C0, C1, One, lower as _dve_lower
from concourse.dve_ops import DveOp, OPS as _DVE_OPS, _SUB_OPCODE_FOR_NAME, CUSTOM_DVE_SPECS, DveOpSpec


def _register_dve(name, spec):
    if name in _SUB_OPCODE_FOR_NAME:
        return next(op for op in _DVE_OPS if op.name == name)
    opcode = max(_SUB_OPCODE_FOR_NAME.values()) + 1
    assert opcode < 0x20
    shas = {}
    for ver in ("v3", "v4"):
        try:
            s = DveOpSpec(name=name, opcode=opcode, uops=_dve_lower(spec, ver=ver), rd1_en=True)
            shas[ver] = s.sha(ver)
        except Exception:
            pass
    op = DveOp(name, spec, subdim=False, uops_sha=shas)
    _DVE_OPS.append(op)
    _SUB_OPCODE_FOR_NAME[name] = opcode
    CUSTOM_DVE_SPECS[name] = spec
    return op


# c' = sig_f * c + sig_i * (2*sig(2g) - 1)
LSTM_C_UPDATE = _register_dve("LSTM_C_UPDATE_ANT", Spec(
    body=C0 * Src0 + C1 * (Src1 + Src1 - One),
    reference=lambda in0, in1, s0, s1: s0 * in0 + s1 * (2.0 * in1 - 1.0),
))


# ---------------------------------------------------------------- program build

def build_program(n_layers=L, debug=False):
    nc = bacc.Bacc("TRN2", target_bir_lowering=False, debug=False, num_devices=N_CORES)

    dt_in = {}
    def din(name, shape, dtype=F32):
        t = nc.dram_tensor(name, list(shape), dtype, kind="ExternalInput")
        dt_in[name] = t
        return t

    ids = din("ids", [S], I32)
    tags = din("tags", [S], I32)
    tags_f = din("tags_f", [S])
    word_emb = din("word_emb", [V, H])
    pos_type = din("pos_type", [S, H])
    emb_lngb = din("emb_lngb", [2, H], BF16)        # g, b
    wqkv = din("wqkv", [n_layers, H, 3 * H], BF16)
    bqkT = din("bqkT", [n_layers, 128, 2 * HC])     # q,k bias feature chunks
    bv_row = din("bv_row", [n_layers, 1, H], BF16)
    wo = din("wo", [n_layers, H, H], BF16)
    bo_row = din("bo_row", [n_layers, 1, H], BF16)
    w1 = din("w1", [n_layers, H, FF], BF16)
    b1_row = din("b1_row", [n_layers, 1, FF], BF16)
    w2 = din("w2", [n_layers, FF, H], BF16)
    b2_row = din("b2_row", [n_layers, 1, H], BF16)
    lngb = din("lngb", [n_layers, 2, 2, H], BF16)   # (ln1,ln2) x (g,b)
    wih1 = din("wih1", [2, H, 4 * R], BF16)         # dir, in, gates (g-cols x2)
    whh1 = din("whh1", [2, R, 4 * R], BF16)
    xb1T = din("xb1T", [2, 128, 4])
    wih2 = din("wih2", [2, 2 * R, 4 * R], BF16)
    whh2 = din("whh2", [2, R, 4 * R], BF16)
    xb2T = din("xb2T", [2, 128, 4])
    clsW = din("clsW", [2 * R, T], BF16)
    clsb = din("clsb", [T, 1])
    crf_trans = din("crf_trans", [T, T])
    crf_start = din("crf_start", [T, 1])
    crf_end = din("crf_end", [T, 1])

    out = nc.dram_tensor("out", [1, 1], F32, kind="ExternalOutput")
    dbg = {}
    if debug:
        dbg["x_out"] = nc.dram_tensor("dbg_x", [S, H], F32, kind="ExternalOutput")
        dbg["hs_out"] = nc.dram_tensor("dbg_hs", [4, R, S], F32, kind="ExternalOutput")
        dbg["em_out"] = nc.dram_tensor("dbg_em", [T, S], F32, kind="ExternalOutput")
        dbg["dn_out"] = nc.dram_tensor("dbg_dn", [1, 4], F32, kind="ExternalOutput")

    with tile.TileContext(nc) as tc:
        _build_body(nc, tc, dt_in, out, dbg, n_layers)
    nc.compile()
    return nc


def _build_body(nc, tc, t_in, t_out, dbg, n_layers):
    ctx_pools = {}

    def pool(name, bufs, space="SBUF"):
        cm = tc.tile_pool(name=name, bufs=bufs, space=space)
        p = cm.__enter__()
        ctx_pools[name] = cm
        return p

    const = pool("const", 1)
    wpool = pool("wts", 1)
    w2pool = pool("w2c", 3)
    act = pool("acts", 1)
    sc = pool("scratch", 2)
    lnp = pool("lnp", 1)
    ps256 = pool("ps256", 3, space="PSUM")
    dram = pool("dram", 2, space="DRAM")
    psum1 = pool("psum1", 1, space="PSUM")
    ps384 = pool("ps384", 4, space="PSUM")

    ident = const.tile([128, 128], BF16)
    make_identity(nc, ident[:])
    ident32 = const.tile([128, 128], F32)
    make_identity(nc, ident32[:])
    ones_col = const.tile([128, 1], BF16)
    nc.vector.memset(ones_col[:], 1.0)
    ones_row = const.tile([1, 128], BF16)
    nc.vector.memset(ones_row[:], 1.0)
    ones32 = const.tile([128, 1], F32)
    nc.vector.memset(ones32[:], 1.0)

    # ---------------- embedding ----------------
    ids_sb = const.tile([128, TC], I32)
    nc.sync.dma_start(ids_sb[:], t_in["ids"].ap().rearrange("(c p) -> p c", p=128))
    x = []          # token-major residual fp32, per token chunk
    for tcn in range(TC):
        xt = act.tile([128, H], F32, tag=f"xres{tcn}", name=f"xres{tcn}")
        nc.gpsimd.indirect_dma_start(
            out=xt[:], out_offset=None, in_=t_in["word_emb"].ap(),
            in_offset=bass.IndirectOffsetOnAxis(ap=ids_sb[:, tcn:tcn + 1], axis=0))
        pt = sc.tile([128, H], F32, tag="pt", name="pt")
        nc.sync.dma_start(pt[:], t_in["pos_type"].ap()[tcn * 128:(tcn + 1) * 128, :])
        nc.vector.tensor_tensor(xt[:], xt[:], pt[:], op=OP.add)
        x.append(xt)

    # LN helper: y_tiles (token-major fp32) -> x_tiles (normalised), using g/b rows
    def layernorm(y_tiles, g_bc, b_bc, out_tag):
        outs = []
        for i, y in enumerate(y_tiles):
            mu = sc.tile([128, 1], F32, tag="mu", name="mu")
            nc.vector.tensor_reduce(mu[:], y[:], axis=mybir.AxisListType.X, op=OP.add)
            nc.vector.tensor_scalar_mul(mu[:], mu[:], 1.0 / H)
            xm = sc.tile([128, H], F32, tag="xm", name="xm")
            nc.vector.tensor_scalar_sub(xm[:], y[:], mu[:, 0:1])
            sq = sc.tile([128, H], F32, tag="sq", name="sq", bufs=1)
            nvar = sc.tile([128, 1], F32, tag="nvar", name="nvar")
            nc.scalar.activation(sq[:], xm[:], AF.Square, accum_out=nvar[:])
            nc.vector.tensor_scalar_mul(nvar[:], nvar[:], 1.0 / H)
            nc.vector.tensor_scalar_add(nvar[:], nvar[:], 1e-12)
            rstd = sc.tile([128, 1], F32, tag="rstd", name="rstd")
            nc.scalar.activation(rstd[:], nvar[:], AF.Sqrt)
            nc.vector.reciprocal(rstd[:], rstd[:])
            o = act.tile([128, H], F32, tag=f"{out_tag}{i}", name=f"{out_tag}{i}")
            nc.vector.scalar_tensor_tensor(o[:], xm[:], rstd[:, 0:1], g_bc[:],
                                           op0=OP.mult, op1=OP.mult)
            nc.vector.tensor_tensor(o[:], o[:], b_bc[:], op=OP.add)
            outs.append(o)
        return outs

    def load_gb(src_ap):   # DRAM row [H] -> [128, H] broadcast
        g = lnp.tile([128, H], BF16, tag="gbc", name="gbc", bufs=2)
        nc.sync.dma_start(g[:], src_ap)
        return g

    def bcast_row(row_ap_2d):
        # row_ap_2d: DRAM AP [1, H]; broadcast over partitions
        return row_ap_2d.rearrange("one h -> one h").to_broadcast([128, H])

    embg = load_gb(t_in["emb_lngb"].ap()[0:1, :].to_broadcast([128, H]))
    embb = load_gb(t_in["emb_lngb"].ap()[1:2, :].to_broadcast([128, H]))
    x = layernorm(x, embg, embb, "xres2")

    # transpose-cast: token-major fp32 [2][128, H] -> feature-major bf16 [128, HC, 256]
    def to_featT(x_tiles, tag):
        xT = act.tile([128, HC, S], BF16, tag=tag, name=tag)
        for tcn, xt in enumerate(x_tiles):
            xb = sc.tile([128, H], BF16, tag="xcast", name="xcast")
            nc.vector.tensor_copy(xb[:], xt[:])
            nc.sync.dma_start_transpose(xT[:, :, tcn * 128:(tcn + 1) * 128], xb[:])
        return xT

    xT = to_featT(x, "xT")

    # ---------------- BERT layers ----------------
    for l in range(n_layers):
        ln1g = load_gb(t_in["lngb"].ap()[l, 0, 0:1, :].to_broadcast([128, H]))
        ln1b = load_gb(t_in["lngb"].ap()[l, 0, 1:2, :].to_broadcast([128, H]))
        ln2g = load_gb(t_in["lngb"].ap()[l, 1, 0:1, :].to_broadcast([128, H]))
        ln2b = load_gb(t_in["lngb"].ap()[l, 1, 1:2, :].to_broadcast([128, H]))

        wq = wpool.tile([128, HC, 3 * H], BF16, tag="wqkv", name="wqkv")
        nc.sync.dma_start(wq[:], t_in["wqkv"].ap()[l].rearrange("(c p) n -> p c n", p=128))
        bqk = wpool.tile([128, 2 * HC], F32, tag="bqk", name="bqk")
        nc.sync.dma_start(bqk[:], t_in["bqkT"].ap()[l])
        bvr = wpool.tile([1, H], BF16, tag="bvr", name="bvr")
        nc.sync.dma_start(bvr[:], t_in["bv_row"].ap()[l])
        wot = wpool.tile([128, HC, H], BF16, tag="wo", name="wo")
        nc.sync.dma_start(wot[:], t_in["wo"].ap()[l].rearrange("(c p) n -> p c n", p=128))
        bor = wpool.tile([1, H], BF16, tag="bor", name="bor")
        nc.sync.dma_start(bor[:], t_in["bo_row"].ap()[l])
        b1r = wpool.tile([1, FF], BF16, tag="b1r", name="b1r")
        nc.sync.dma_start(b1r[:], t_in["b1_row"].ap()[l])
        b2r = wpool.tile([1, H], BF16, tag="b2r", name="b2r")
        nc.sync.dma_start(b2r[:], t_in["b2_row"].ap()[l])

        # --- QKV (q,k feature-major; v token-major)
        qkT = act.tile([128, 2 * HC, S], BF16, tag="qkT", name="qkT")
        for mc in range(2 * HC):
            pq = ps256.tile([128, S], F32, tag="ps256", name="ps256")
            for kc in range(HC):
                nc.tensor.matmul(pq[:], wq[:, kc, mc * 128:(mc + 1) * 128],
                                 xT[:, kc, :], start=(kc == 0), stop=(kc == HC - 1))
            if mc % 2 == 0:
                nc.vector.tensor_scalar_add(qkT[:, mc, :], pq[:], bqk[:, mc:mc + 1])
            else:
                nc.scalar.activation(qkT[:, mc, :], pq[:], AF.Identity, bias=bqk[:, mc:mc + 1])
        vtm = act.tile([128, TC, H], BF16, tag="vtm", name="vtm")
        for tcn in range(TC):
            for nb in range(2):
                pv = ps384.tile([128, 384], F32, tag="ps384", name="ps384")
                for kc in range(HC):
                    nc.tensor.matmul(pv[:], xT[:, kc, tcn * 128:(tcn + 1) * 128],
                                     wq[:, kc, 2 * H + nb * 384: 2 * H + (nb + 1) * 384],
                                     start=(kc == 0), stop=False)
                nc.tensor.matmul(pv[:], ones_row[:],
                                 bvr[:, nb * 384:(nb + 1) * 384], start=False, stop=True)
                if nb % 2 == 0:
                    nc.vector.tensor_copy(vtm[:, tcn, nb * 384:(nb + 1) * 384], pv[:])
                else:
                    nc.scalar.copy(vtm[:, tcn, nb * 384:(nb + 1) * 384], pv[:])

        # --- attention: scores/exp/sums
        expT = []
        for kc2 in range(TC):
            e = act.tile([128, NH, S], BF16, tag=f"expT{kc2}", name=f"expT{kc2}")
            expT.append(e)
        s_all = act.tile([1, NH, S], F32, tag="s_all", name="s_all")
        for h in range(NH):
            base = (h % 2) * 64
            chunk = h // 2
            psum_sum = psum1.tile([1, S], F32, tag="psum1", name="psum1")
            for kc2 in range(TC):
                pss = ps256.tile([128, S], F32, tag="ps256", name="pss")
                nc.tensor.matmul(
                    pss[:],
                    qkT[base:base + 64, HC + chunk, kc2 * 128:(kc2 + 1) * 128],
                    qkT[base:base + 64, chunk, :], start=True, stop=True)
                nc.scalar.activation(expT[kc2][:, h, :], pss[:], AF.Exp, scale=0.125)
                nc.tensor.matmul(psum_sum[:], ones_col[:], expT[kc2][:, h, :],
                                 start=(kc2 == 0), stop=(kc2 == TC - 1))
            if h % 2 == 0:
                nc.vector.tensor_copy(s_all[:, h, :], psum_sum[:])
            else:
                nc.scalar.copy(s_all[:, h, :], psum_sum[:])

        # --- reciprocal path (via DRAM for the partition broadcast)
        sdram = dram.tile([NH * S], F32, tag="sdram", name="sdram")
        nc.sync.dma_start(sdram[:].rearrange("(h q) -> h q", h=NH), s_all[:, :, :])
        s_sp = sc.tile([128, NH * TC], F32, tag="s_sp", name="s_sp")
        nc.sync.dma_start(
            s_sp[:].rearrange("p (h c) -> p h c", h=NH),
            sdram[:].rearrange("(h c p) -> p h c", p=128, h=NH))
        nc.vector.reciprocal(s_sp[:], s_sp[:])
        s_bf = sc.tile([128, NH * TC], BF16, tag="s_bf", name="s_bf")
        nc.vector.tensor_copy(s_bf[:], s_sp[:])
        rdram = dram.tile([NH * S], BF16, tag="rdram", name="rdram")
        nc.sync.dma_start(
            rdram[:].rearrange("(h c p) -> p h c", p=128, h=NH),
            s_bf[:].rearrange("p (h c) -> p h c", h=NH))
        Rt = act.tile([128, NH, S], BF16, tag="Rt", name="Rt")
        nc.sync.dma_start(Rt[:], rdram[:].rearrange("(h q) -> h q", h=NH)[None, :, :].to_broadcast([128, NH, S]))

        # --- ctx (unscaled bf16) then scale
        ctxu = act.tile([128, HC, S], BF16, tag="ctxu", name="ctxu")
        for h in range(NH):
            base = (h % 2) * 64
            chunk = h // 2
            pc = ps256.tile([128, S], F32, tag="ps256", name="pc")
            for kc2 in range(TC):
                nc.tensor.matmul(pc[:64, :], vtm[:, kc2, h * 64:(h + 1) * 64],
                                 expT[kc2][:, h, :], start=(kc2 == 0), stop=(kc2 == TC - 1))
            if h % 2 == 0:
                nc.vector.tensor_copy(ctxu[base:base + 64, chunk, :], pc[:64, :])
            else:
                nc.scalar.copy(ctxu[base:base + 64, chunk, :], pc[:64, :])
        ctxT = act.tile([128, HC, S], BF16, tag="ctxT", name="ctxT")
        for chunk in range(HC):
            nc.vector.tensor_tensor(
                ctxT[0:64, chunk, :], ctxu[0:64, chunk, :], Rt[0:64, 2 * chunk, :], op=OP.mult)
            nc.vector.tensor_tensor(
                ctxT[64:128, chunk, :], ctxu[64:128, chunk, :], Rt[64:128, 2 * chunk + 1, :], op=OP.mult)

        # --- Wo + residual + LN1
        y1 = []
        for tcn in range(TC):
            yt = act.tile([128, H], F32, tag=f"y{tcn}", name=f"y1{tcn}")
            for nb in range(2):
                pao = ps384.tile([128, 384], F32, tag="ps384", name="pao")
                for dc in range(HC):
                    nc.tensor.matmul(pao[:], ctxT[:, dc, tcn * 128:(tcn + 1) * 128],
                                     wot[:, dc, nb * 384:(nb + 1) * 384],
                                     start=(dc == 0), stop=False)
                nc.tensor.matmul(pao[:], ones_row[:], bor[:, nb * 384:(nb + 1) * 384],
                                 start=False, stop=True)
                nc.vector.tensor_tensor(yt[:, nb * 384:(nb + 1) * 384], pao[:],
                                        x[tcn][:, nb * 384:(nb + 1) * 384], op=OP.add)
            y1.append(yt)
        x = layernorm(y1, ln1g, ln1b, "xres")
        xT2 = to_featT(x, "xT2")

        # --- FFN (token-major hdn blocks, DMA-transposed to feature-major)
        hdnT = act.tile([128, FC, S], BF16, tag="hdnT", name="hdnT")
        for nb in range(FF // 384):
            w1cs = []
            for kc in range(HC):
                w1c = w2pool.tile([128, 384], BF16, tag="w1c", name="w1c", bufs=7)
                nc.sync.dma_start(w1c[:], t_in["w1"].ap()[l, kc * 128:(kc + 1) * 128,
                                                          nb * 384:(nb + 1) * 384])
                w1cs.append(w1c)
            for tcn in range(TC):
                ph = ps384.tile([128, 384], F32, tag="ps384", name="ph")
                for kc in range(HC):
                    nc.tensor.matmul(ph[:], xT2[:, kc, tcn * 128:(tcn + 1) * 128],
                                     w1cs[kc][:], start=(kc == 0), stop=False)
                nc.tensor.matmul(ph[:], ones_row[:], b1r[:, nb * 384:(nb + 1) * 384],
                                 start=False, stop=True)
                hdn_c = sc.tile([128, 384], BF16, tag="hdnc", name="hdnc", bufs=3)
                nc.scalar.activation(hdn_c[:], ph[:], AF.Gelu)
                nc.sync.dma_start_transpose(
                    hdnT[:, nb * 3:(nb + 1) * 3, tcn * 128:(tcn + 1) * 128], hdn_c[:])
        pfs = [[ps384.tile([128, 384], F32, tag="ps384", name=f"pf{t}{n}")
                for n in range(2)] for t in range(TC)]
        for fc in range(FC):
            w2c = w2pool.tile([128, H], BF16, tag="w2c", name="w2c")
            nc.sync.dma_start(w2c[:], t_in["w2"].ap()[l, fc * 128:(fc + 1) * 128, :])
            for tcn in range(TC):
                for nb in range(2):
                    nc.tensor.matmul(pfs[tcn][nb][:], hdnT[:, fc, tcn * 128:(tcn + 1) * 128],
                                     w2c[:, nb * 384:(nb + 1) * 384],
                                     start=(fc == 0), stop=False)
        y2 = []
        for tcn in range(TC):
            yt = act.tile([128, H], F32, tag=f"y{tcn}", name=f"y2{tcn}")
            for nb in range(2):
                nc.tensor.matmul(pfs[tcn][nb][:], ones_row[:], b2r[:, nb * 384:(nb + 1) * 384],
                                 start=False, stop=True)
                nc.vector.tensor_tensor(yt[:, nb * 384:(nb + 1) * 384], pfs[tcn][nb][:],
                                        x[tcn][:, nb * 384:(nb + 1) * 384], op=OP.add)
            y2.append(yt)
        x = layernorm(y2, ln2g, ln2b, "xres2")
        xT = to_featT(x, "xT")

    if dbg:
        for tcn in range(TC):
            nc.sync.dma_start(dbg["x_out"].ap()[tcn * 128:(tcn + 1) * 128, :], x[tcn][:])

    # free BERT-only psum pools before the LSTM phase (LIFO order)
    for pname in ("ps384", "psum1"):
        ctx_pools.pop(pname).__exit__(None, None, None)


    # ---------------- BiLSTM ----------------
    # xg precompute helper: out_sb [128, 4, S] fp32 from contraction tiles
    def xg_precompute(w_ap, nchunks, rhs_fn, bias_tile, tag):
        xg = act.tile([128, 4, S], BF16, tag="xg" + tag[-1], name=tag)
        wt = wpool.tile([128, nchunks, 4 * R], BF16, tag=f"w{tag}", name=f"w{tag}")
        nc.sync.dma_start(wt[:], w_ap.rearrange("(c p) n -> p c n", p=128))
        for gc in range(4):
            pg = ps256.tile([128, S], F32, tag="ps256", name="pg")
            for kc in range(nchunks):
                nc.tensor.matmul(pg[:], wt[:, kc, gc * 128:(gc + 1) * 128], rhs_fn(kc),
                                 start=(kc == 0), stop=(kc == nchunks - 1))
            nc.scalar.activation(xg[:, gc, :], pg[:], AF.Identity, bias=bias_tile[:, gc:gc + 1])
        return xg

    xb1 = const.tile([128, 4, 2], F32)
    nc.sync.dma_start(xb1[:], t_in["xb1T"].ap().rearrange("d p g -> p g d"))
    xb2 = const.tile([128, 4, 2], F32)
    nc.sync.dma_start(xb2[:], t_in["xb2T"].ap().rearrange("d p g -> p g d"))

    whh1t = const.tile([128, 2, 4 * R], BF16)
    nc.sync.dma_start(whh1t[:], t_in["whh1"].ap().rearrange("d p n -> p d n"))
    whh2t = const.tile([128, 2, 4 * R], BF16)
    nc.sync.dma_start(whh2t[:], t_in["whh2"].ap().rearrange("d p n -> p d n"))

    def lstm_layer(xg_f, xg_b, whht, hs_tag):
        # preload XG psum (2 banks per dir), run both chains interleaved
        XGs = []
        hss = []
        xg_pools = []
        for d, xg in ((0, xg_f), (1, xg_b)):
            XGcm = tc.tile_pool(name=f"XG{hs_tag}{d}", bufs=1, space="PSUM")
            XG = XGcm.__enter__()
            xg_pools.append(XGcm)
            X = XG.tile([128, S, 4], F32, tag=f"XG{d}", name=f"XG{d}")
            for b in range(S * 4 // 512):
                nc.tensor.matmul(X[:, b * 128:(b + 1) * 128, :], ident[:],
                                 xg[:, :, b * 128:(b + 1) * 128].transpose([0, 2, 1]),
                                 start=True, stop=False, skip_group_check=True)
            XGs.append(X)
            hs = act.tile([128, S + 1], BF16, tag=f"hs{hs_tag}{d}", name=f"hs{hs_tag}{d}")
            nc.vector.memset(hs[:, 0:1], 0.0)
            hss.append(hs)
        cs = [sc.tile([128, 1], F32, tag=f"c{d}", name=f"c{d}") for d in range(2)]
        for d in range(2):
            nc.vector.memset(cs[d][:], 0.0)
        for t in range(S):
            for d in range(2):
                X, hs, c = XGs[d], hss[d], cs[d]
                for j in range(4):
                    nc.tensor.matmul(X[:, t, j:j + 1], whht[:, d, j * R:(j + 1) * R],
                                     hs[:, t:t + 1], start=False,
                                     stop=(t == S - 1 and j == 3), skip_group_check=True)
                sig = sc.tile([128, 4], F32, tag=f"sig{d}", name=f"sig{d}", bufs=3)
                nc.scalar.activation(sig[:], X[:, t, :], AF.Sigmoid)
                nc.vector._custom_dve(LSTM_C_UPDATE, out=c[:], in0=c[:], in1=sig[:, 2:3],
                                      s0=sig[:, 1:2], s1=sig[:, 0:1])
                tch = sc.tile([128, 1], F32, tag=f"tch{d}", name=f"tch{d}", bufs=3)
                nc.scalar.activation(tch[:], c[:], AF.Tanh)
                nc.vector.tensor_tensor(hs[:, t + 1:t + 2], sig[:, 3:4], tch[:], op=OP.mult)
        for XGp in reversed(xg_pools):
            XGp.__exit__(None, None, None)
        return hss

    # layer 1: forward dir reads xT natural, backward reads xT time-reversed
    def rev(ap):   # reverse the last (time) axis of [128, S] AP
        return ap[:, ::-1]

    xg1f = xg_precompute(t_in["wih1"].ap()[0], HC, lambda kc: xT[:, kc, :], xb1[:, :, 0], "xg1f")
    xg1b = xg_precompute(t_in["wih1"].ap()[1], HC, lambda kc: rev(xT[:, kc, :]), xb1[:, :, 1], "xg1b")
    hs1 = lstm_layer(xg1f, xg1b, whh1t, "1")

    # layer 2: input = [f1; b1]; forward: f natural + b reversed; backward: f reversed + b natural
    xg2f = xg_precompute(
        t_in["wih2"].ap()[0], 2,
        lambda kc: hs1[0][:, 1:S + 1] if kc == 0 else rev(hs1[1][:, 1:S + 1]),
        xb2[:, :, 0], "xg2f")
    xg2b = xg_precompute(
        t_in["wih2"].ap()[1], 2,
        lambda kc: rev(hs1[0][:, 1:S + 1]) if kc == 0 else hs1[1][:, 1:S + 1],
        xb2[:, :, 1], "xg2b")
    hs2 = lstm_layer(xg2f, xg2b, whh2t, "2")

    if dbg:
        for i, hsx in enumerate(hs1 + hs2):
            h32 = sc.tile([128, S], F32, tag="h32", name="h32")
            nc.vector.tensor_copy(h32[:], hsx[:, 1:S + 1])
            nc.sync.dma_start(dbg["hs_out"].ap()[i], h32[:])

    # ---------------- classifier ----------------
    clsw = const.tile([128, 2, T], BF16)
    nc.sync.dma_start(clsw[:], t_in["clsW"].ap().rearrange("(c p) t -> p c t", p=128))
    clsb_sb = const.tile([T, 1], F32)
    nc.sync.dma_start(clsb_sb[:], t_in["clsb"].ap())
    pem = ps256.tile([T, S], F32, tag="ps256", name="pem")
    nc.tensor.matmul(pem[:], clsw[:, 0, :], hs2[0][:, 1:S + 1], start=True, stop=False)
    nc.tensor.matmul(pem[:], clsw[:, 1, :], rev(hs2[1][:, 1:S + 1]), start=False, stop=True)
    emT = const.tile([T, S], F32)
    nc.scalar.activation(emT[:], pem[:], AF.Identity, bias=clsb_sb[:, 0:1])
    if dbg:
        nc.sync.dma_start(dbg["em_out"].ap(), emT[:])

    # ---------------- CRF ----------------
    trans_sb = const.tile([T, T], F32)
    nc.sync.dma_start(trans_sb[:], t_in["crf_trans"].ap())
    start_sb = const.tile([T, 1], F32)
    nc.sync.dma_start(start_sb[:], t_in["crf_start"].ap())
    end_sb = const.tile([T, 1], F32)
    nc.sync.dma_start(end_sb[:], t_in["crf_end"].ap())
    expM = const.tile([T, T], F32)
    nc.scalar.activation(expM[:], trans_sb[:], AF.Exp)
    expEm = const.tile([T, S], F32)
    nc.scalar.activation(expEm[:], emT[:], AF.Exp)

    logs = const.tile([1, 64], F32)   # renorm log collector
    nc.vector.memset(logs[:], 0.0)
    n_logs = [0]
    psc = pool("psc", 2, space="PSUM")

    def log_and_renorm(vec, width, tag):
        # vec [T, width] sbuf; compute total sum -> logs[n], scale vec by 1/sum
        pssum = psc.tile([1, T], F32, tag="cs", name="cs", bufs=1)
        nc.tensor.matmul(pssum[:, :width] if width < T else pssum[:],
                         ones32[:T, :], vec[:], start=True, stop=True)
        tot = sc.tile([1, 1], F32, tag="tot", name="tot")
        nc.vector.tensor_reduce(tot[:], pssum[:, :width] if width < T else pssum[:],
                                axis=mybir.AxisListType.X, op=OP.add)
        nc.scalar.activation(logs[:, n_logs[0]:n_logs[0] + 1], tot[:], AF.Ln)
        n_logs[0] += 1
        rec = sc.tile([1, 1], F32, tag="rec", name="rec")
        nc.vector.reciprocal(rec[:], tot[:])
        recb = sc.tile([T, 1], F32, tag="recb", name="recb")
        nc.gpsimd.partition_broadcast(recb[:], rec[:])
        nc.vector.tensor_scalar_mul(vec[:], vec[:], recb[:, 0:1])

    HALF = S // 2
    # chain A: probability vector scan over t = 1..HALF-1 (p0 at t=0)
    p_vec = const.tile([T, 1], F32)
    nc.scalar.activation(p_vec[:], emT[:, 0:1], AF.Exp, bias=start_sb[:, 0:1])
    for t in range(1, HALF):
        pp = psc.tile([T, 1], F32, tag="pp", name="pp")
        nc.tensor.matmul(pp[:], expM[:], p_vec[:], start=True, stop=True)
        nc.vector.tensor_tensor(p_vec[:], pp[:], expEm[:, t:t + 1], op=OP.mult)
        if t % RENORM == 0:
            log_and_renorm(p_vec, 1, "pA")

    # chain B: S_t = Mtilde_t^T . S_{t-1}, t = HALF..S-1 ; Mtilde precomputed
    emB = const.tile([T, S - HALF, T], F32)   # emB[k, t, j] = expEm[j, HALF+t] (bcast over k)
    emdram = dram.tile([T * (S - HALF)], F32, tag="emd", name="emd")
    nc.sync.dma_start(emdram[:].rearrange("(t j) -> j t", j=T), expEm[:, HALF:S])
    nc.sync.dma_start(emB[:], emdram[:].rearrange("(t j) -> t j", j=T)[None, :, :].to_broadcast([T, S - HALF, T]))
    Mt = const.tile([T, S - HALF, T], F32)
    nc.vector.tensor_tensor(Mt[:], emB[:], expM[:][:, None, :].to_broadcast([T, S - HALF, T]), op=OP.mult)
    S_mat = const.tile([T, T], F32)
    nc.vector.tensor_copy(S_mat[:], ident32[:T, :T])
    for t in range(S - HALF):
        ps_ = psc.tile([T, T], F32, tag="pp", name="ppS")
        nc.tensor.matmul(ps_[:], Mt[:, t, :], S_mat[:], start=True, stop=True)
        if (t + 1) % RENORM == 0:
            nc.vector.tensor_copy(S_mat[:], ps_[:])
            log_and_renorm(S_mat, T, "SB")
        else:
            nc.vector.tensor_copy(S_mat[:], ps_[:])

    # combine: p_final = S_final^T @ p_mid ; denom = ln(sum_j p_final * exp(end)) + sum(logs)
    pSt = psc.tile([T, T], F32, tag="pp", name="pSt")
    nc.tensor.transpose(pSt[:], S_mat[:], ident32[:T, :T])
    St_T = const.tile([T, T], F32)
    nc.vector.tensor_copy(St_T[:], pSt[:])
    pfin = psc.tile([T, 1], F32, tag="pp", name="pfin")
    nc.tensor.matmul(pfin[:], St_T[:], p_vec[:], start=True, stop=True)
    expEnd = const.tile([T, 1], F32)
    nc.scalar.activation(expEnd[:], end_sb[:], AF.Exp)
    pfe = const.tile([T, 1], F32)
    nc.vector.tensor_tensor(pfe[:], pfin[:], expEnd[:], op=OP.mult)
    pden = psc.tile([1, T], F32, tag="cs", name="pden", bufs=1)
    nc.tensor.matmul(pden[:, 0:1], ones32[:T, :], pfe[:], start=True, stop=True)
    denom = const.tile([1, 1], F32)
    nc.scalar.activation(denom[:], pden[:, 0:1], AF.Ln)
    logsum = const.tile([1, 1], F32)
    nc.vector.tensor_reduce(logsum[:], logs[:], axis=mybir.AxisListType.X, op=OP.add)
    nc.vector.tensor_tensor(denom[:], denom[:], logsum[:], op=OP.add)

    # ---------------- numerator ----------------
    tags_b = const.tile([T, S], F32)
    nc.sync.dma_start(tags_b[:], t_in["tags_f"].ap()[None, :].to_broadcast([T, S]))
    iota_c = const.tile([T, 1], I32)
    nc.gpsimd.iota(iota_c[:], pattern=[[0, 1]], base=0, channel_multiplier=1)
    iota_f = const.tile([T, 1], F32)
    nc.vector.tensor_copy(iota_f[:], iota_c[:])
    onehot = const.tile([T, S], F32)
    nc.vector.tensor_scalar(onehot[:], tags_b[:], iota_f[:, 0:1], None,
                            op0=OP.is_equal)
    # em-gold: sum over (t,s) of emT*onehot ; start/end-gold via onehot cols
    emoh = const.tile([T, S], F32)
    gold1 = const.tile([T, 1], F32)
    nc.vector.tensor_tensor(emoh[:], emT[:], onehot[:], op=OP.mult)
    nc.vector.tensor_reduce(gold1[:], emoh[:], axis=mybir.AxisListType.X, op=OP.add)
    seg = const.tile([T, 1], F32)
    nc.vector.tensor_tensor(seg[:], start_sb[:], onehot[:, 0:1], op=OP.mult)
    nc.vector.tensor_tensor(gold1[:], gold1[:], seg[:], op=OP.add)
    nc.vector.tensor_tensor(seg[:], end_sb[:], onehot[:, S - 1:S], op=OP.mult)
    nc.vector.tensor_tensor(gold1[:], gold1[:], seg[:], op=OP.add)
    # trans-gold: A = trans^T-sel: A[j, s] = trans[tag_s, j] = sum_i trans[i,j]*onehot[i,s]
    pA = psc.tile([T, S], F32, tag="pAo", name="pA", bufs=1)
    nc.tensor.matmul(pA[:, 0:S - 1], trans_sb[:], onehot[:, 0:S - 1], start=True, stop=True)
    tg = const.tile([T, S], F32)
    nc.vector.tensor_tensor(tg[:, 0:S - 1], pA[:, 0:S - 1], onehot[:, 1:S], op=OP.mult)
    tgs = const.tile([T, 1], F32)
    nc.vector.tensor_reduce(tgs[:], tg[:, 0:S - 1], axis=mybir.AxisListType.X, op=OP.add)
    nc.vector.tensor_tensor(gold1[:], gold1[:], tgs[:], op=OP.add)
    pnum = psc.tile([1, T], F32, tag="cs", name="pnum", bufs=1)
    nc.tensor.matmul(pnum[:, 0:1], ones32[:T, :], gold1[:], start=True, stop=True)

    # partial = 0.5 * (denom - num)
    part = const.tile([1, 1], F32)
    nc.vector.tensor_tensor(part[:], denom[:], pnum[:, 0:1], op=OP.subtract)
    nc.vector.tensor_scalar_mul(part[:], part[:], 0.5)
    if dbg:
        dn = const.tile([1, 4], F32)
        nc.vector.tensor_copy(dn[:, 0:1], denom[:])
        nc.vector.tensor_copy(dn[:, 1:2], pnum[:, 0:1])
        nc.vector.tensor_copy(dn[:, 2:3], logsum[:])
        nc.vector.tensor_copy(dn[:, 3:4], part[:])
        nc.sync.dma_start(dbg["dn_out"].ap(), dn[:])

    # ---------------- final AllReduce ----------------
    bin_ = dram.tile([1, 1], F32, tag="arin", name="arin")
    bout = dram.tile([1, 1], F32, tag="arout", name="arout")
    nc.sync.dma_start(bin_[:], part[:])
    nc.gpsimd.collective_compute(
        "AllReduce", OP.add, replica_groups=[list(range(N_CORES))],
        ins=[bin_[:].opt()], outs=[bout[:].opt()])
    nc.sync.dma_start(t_out.ap(), bout[:])

    for p in reversed(list(ctx_pools.values())):
        p.__exit__(None, None, None)


# ---------------------------------------------------------------- host prep

def _bf16(a):
    return np.asarray(a, np.float32).astype(ml_dtypes.bfloat16)


def prepare_inputs(input_ids, attention_mask, tags, params, n_layers=L):
    p = params
    lay = p["layers"]
    per_core = []

    wqkv = np.concatenate([lay["Wq"], lay["Wk"], lay["Wv"]], axis=2)  # [L, H, 3H]
    bqk = np.concatenate([lay["bq"][:n_layers], lay["bk"][:n_layers]], axis=1)
    bqkT = bqk.reshape(n_layers, 2 * HC, 128).transpose(0, 2, 1)      # [L, 128, 12]
    lngb = np.stack([
        np.stack([lay["ln1_g"], lay["ln1_b"]], axis=1),
        np.stack([lay["ln2_g"], lay["ln2_b"]], axis=1)], axis=1)      # [L, 2, 2, H]

    def lstm_dir(lp, sfx):
        wih = np.asarray(lp[f"Wih_{sfx}"], np.float32).T.copy()   # [in, 4R]
        whh = np.asarray(lp[f"Whh_{sfx}"], np.float32).T.copy()   # [R, 4R]
        xb = np.asarray(lp[f"bih_{sfx}"], np.float32) + np.asarray(lp[f"bhh_{sfx}"], np.float32)
        wih[:, 2 * R:3 * R] *= 2.0
        whh[:, 2 * R:3 * R] *= 2.0
        xb = xb.copy()
        xb[2 * R:3 * R] *= 2.0
        return wih, whh, xb.reshape(4, R).T    # xbT [128, 4]

    w1f, h1f, b1f = lstm_dir(p["lstm0"], "f")
    w1b, h1b, b1b = lstm_dir(p["lstm0"], "b")
    w2f, h2f, b2f = lstm_dir(p["lstm1"], "f")
    w2b, h2b, b2b = lstm_dir(p["lstm1"], "b")

    shared = {
        "word_emb": np.asarray(p["word_emb"], np.float32),
        "pos_type": np.asarray(p["pos_emb"][:S], np.float32) + np.asarray(p["type_emb"][0], np.float32)[None, :],
        "emb_lngb": _bf16(np.stack([p["emb_ln_g"], p["emb_ln_b"]])),
        "wqkv": _bf16(wqkv[:n_layers]),
        "bqkT": np.ascontiguousarray(bqkT[:n_layers], dtype=np.float32),
        "bv_row": _bf16(lay["bv"][:n_layers, None, :]),
        "wo": _bf16(lay["Wo"][:n_layers]),
        "bo_row": _bf16(lay["bo"][:n_layers, None, :]),
        "w1": _bf16(lay["W1"][:n_layers]),
        "b1_row": _bf16(lay["b1"][:n_layers, None, :]),
        "w2": _bf16(lay["W2"][:n_layers]),
        "b2_row": _bf16(lay["b2"][:n_layers, None, :]),
        "lngb": _bf16(lngb[:n_layers]),
        "wih1": np.stack([_bf16(w1f), _bf16(w1b)]),
        "whh1": np.stack([_bf16(h1f), _bf16(h1b)]),
        "xb1T": np.stack([b1f, b1b]).astype(np.float32),
        "wih2": np.stack([_bf16(w2f), _bf16(w2b)]),
        "whh2": np.stack([_bf16(h2f), _bf16(h2b)]),
        "xb2T": np.stack([b2f, b2b]).astype(np.float32),
        "clsW": _bf16(p["cls_W"]),
        "clsb": np.asarray(p["cls_b"], np.float32)[:, None],
        "crf_trans": np.asarray(p["crf_trans"], np.float32),
        "crf_start": np.asarray(p["crf_start"], np.float32)[:, None],
        "crf_end": np.asarray(p["crf_end"], np.float32)[:, None],
    }
    ids = np.asarray(input_ids, np.int64).astype(np.int32)
    tg = np.asarray(tags, np.int64).astype(np.int32)
    for c in range(N_CORES):
        b = c // 2
        m = dict(shared)
        m["ids"] = np.ascontiguousarray(ids[b])
        m["tags"] = np.ascontiguousarray(tg[b])
        m["tags_f"] = np.ascontiguousarray(tg[b].astype(np.float32))
        per_core.append(m)
    return per_core


_CACHE = {}


def _get_program(n_layers=L, debug=False):
    key = (n_layers, debug)
    if key not in _CACHE:
        _CACHE[key] = build_program(n_layers, debug)
    return _CACHE[key]


def kernel(input_ids, attention_mask, tags, params, n_layers=L, debug=False, trace=False):
    nc = _get_program(n_layers, debug)
    in_maps = prepare_inputs(input_ids, attention_mask, tags, params, n_layers)
    res = run_bass_kernel_spmd(nc, in_maps, list(range(N_CORES)), trace=trace)
    out = np.float32(res.results[0]["out"][0, 0])
    if debug or trace:
        kernel.last_results = res
    return np.asarray(out, dtype=np.float32).reshape(())


# revision 16
# speedup vs baseline: 1.0262x; 1.0262x over previous
"""BERT-base + 2-layer BiLSTM + CRF negative-log-likelihood on 8 Trainium2 NeuronCores.

Sharding: data-parallel over the batch (B=4). Core c computes batch element
c//2 end-to-end (pairs are redundant — zero cross-core traffic until the final
scalar AllReduce, each core contributes (denom-num)/2 of its batch element).

BERT: token-major fp32 residual stream, feature-major bf16 matmul operands
(produced via DMA transpose), fp32 PSUM accumulation.  Softmax skips the max
subtraction (scores are bounded ~|2|) so attention runs in exp space with one
reciprocal pass per layer.  LSTM: recurrent matmuls accumulate onto the
precomputed input gates held in PSUM; all four gate nonlinearities are a
single batched sigmoid (tanh(g) = 2*sigmoid(2g)-1 with the 2x folded into the
weights on the host); the cell update is one fused custom DVE op.  CRF: the
forward recursion runs in probability space (p' = exp(trans)^T p * exp(em_t))
as tiny 9x9 matmuls with periodic renormalisation; the two sequence halves run
as independent chains (vector scan + transposed matrix product) and are joined
at the end.
"""

import sys, os

for _p in ("/opt/trn_rl_repo", os.path.expanduser("~/.axon_site/_ro/trn_rl_repo")):
    if os.path.isdir(_p) and _p not in sys.path:
        sys.path.append(_p)

import numpy as np
import ml_dtypes

import concourse.bass as bass
import concourse.tile as tile
from concourse import mybir, bacc
from concourse.bass_utils import run_bass_kernel_spmd
from concourse.masks import make_identity

AF = mybir.ActivationFunctionType
OP = mybir.AluOpType
BF16 = mybir.dt.bfloat16
F32 = mybir.dt.float32
I32 = mybir.dt.int32

# model dims
B, S, H, L, NH, DH, FF = 4, 256, 768, 12, 12, 64, 3072
V, P, R, T = 30522, 512, 128, 9
TC = S // 128          # token chunks
HC = H // 128          # hidden chunks
FC = FF // 128         # ff chunks
N_CORES = 8
RENORM = 16            # CRF renormalisation interval

# ---------------------------------------------------------------- custom DVE op
from concourse.dve_spec import Spec, Src0, Src1, C0, C1, One, lower as _dve_lower
from concourse.dve_ops import DveOp, OPS as _DVE_OPS, _SUB_OPCODE_FOR_NAME, CUSTOM_DVE_SPECS, DveOpSpec


def _register_dve(name, spec):
    if name in _SUB_OPCODE_FOR_NAME:
        return next(op for op in _DVE_OPS if op.name == name)
    opcode = max(_SUB_OPCODE_FOR_NAME.values()) + 1
    assert opcode < 0x20
    shas = {}
    for ver in ("v3", "v4"):
        try:
            s = DveOpSpec(name=name, opcode=opcode, uops=_dve_lower(spec, ver=ver), rd1_en=True)
            shas[ver] = s.sha(ver)
        except Exception:
            pass
    op = DveOp(name, spec, subdim=False, uops_sha=shas)
    _DVE_OPS.append(op)
    _SUB_OPCODE_FOR_NAME[name] = opcode
    CUSTOM_DVE_SPECS[name] = spec
    return op


# c' = sig_f * c + sig_i * (2*sig(2g) - 1)
LSTM_C_UPDATE = _register_dve("LSTM_C_UPDATE_ANT", Spec(
    body=C0 * Src0 + C1 * (Src1 + Src1 - One),
    reference=lambda in0, in1, s0, s1: s0 * in0 + s1 * (2.0 * in1 - 1.0),
))


# ---------------------------------------------------------------- program build

def build_program(n_layers=L, debug=False):
    nc = bacc.Bacc("TRN2", target_bir_lowering=False, debug=False, num_devices=N_CORES)

    dt_in = {}
    def din(name, shape, dtype=F32):
        t = nc.dram_tensor(name, list(shape), dtype, kind="ExternalInput")
        dt_in[name] = t
        return t

    ids = din("ids", [S], I32)
    tags = din("tags", [S], I32)
    tags_f = din("tags_f", [S])
    word_emb = din("word_emb", [V, H])
    pos_type = din("pos_type", [S, H])
    emb_lngb = din("emb_lngb", [2, H], BF16)        # g, b
    wqkv = din("wqkv", [n_layers, H, 3 * H], BF16)
    bqkT = din("bqkT", [n_layers, 128, 2 * HC])     # q,k bias feature chunks
    bv_row = din("bv_row", [n_layers, 1, H], BF16)
    wo = din("wo", [n_layers, H, H], BF16)
    bo_row = din("bo_row", [n_layers, 1, H], BF16)
    w1 = din("w1", [n_layers, H, FF], BF16)
    b1_row = din("b1_row", [n_layers, 1, FF], BF16)
    w2 = din("w2", [n_layers, FF, H], BF16)
    b2_row = din("b2_row", [n_layers, 1, H], BF16)
    lngb = din("lngb", [n_layers, 2, 2, H], BF16)   # (ln1,ln2) x (g,b)
    wih1 = din("wih1", [2, H, 4 * R], BF16)         # dir, in, gates (g-cols x2)
    whh1 = din("whh1", [2, R, 4 * R], BF16)
    xb1T = din("xb1T", [2, 128, 4])
    wih2 = din("wih2", [2, 2 * R, 4 * R], BF16)
    whh2 = din("whh2", [2, R, 4 * R], BF16)
    xb2T = din("xb2T", [2, 128, 4])
    clsW = din("clsW", [2 * R, T], BF16)
    clsb = din("clsb", [T, 1])
    crf_trans = din("crf_trans", [T, T])
    crf_start = din("crf_start", [T, 1])
    crf_end = din("crf_end", [T, 1])

    out = nc.dram_tensor("out", [1, 1], F32, kind="ExternalOutput")
    dbg = {}
    if debug:
        dbg["x_out"] = nc.dram_tensor("dbg_x", [S, H], F32, kind="ExternalOutput")
        dbg["hs_out"] = nc.dram_tensor("dbg_hs", [4, R, S], F32, kind="ExternalOutput")
        dbg["em_out"] = nc.dram_tensor("dbg_em", [T, S], F32, kind="ExternalOutput")
        dbg["dn_out"] = nc.dram_tensor("dbg_dn", [1, 4], F32, kind="ExternalOutput")

    with tile.TileContext(nc) as tc:
        _build_body(nc, tc, dt_in, out, dbg, n_layers)
    nc.compile()
    return nc


def _build_body(nc, tc, t_in, t_out, dbg, n_layers):
    ctx_pools = {}

    def pool(name, bufs, space="SBUF"):
        cm = tc.tile_pool(name=name, bufs=bufs, space=space)
        p = cm.__enter__()
        ctx_pools[name] = cm
        return p

    const = pool("const", 1)
    wpool = pool("wts", 1)
    w2pool = pool("w2c", 3)
    act = pool("acts", 1)
    sc = pool("scratch", 2)
    lnp = pool("lnp", 1)
    ps256 = pool("ps256", 3, space="PSUM")
    dram = pool("dram", 2, space="DRAM")
    psum1 = pool("psum1", 1, space="PSUM")
    ps384 = pool("ps384", 4, space="PSUM")

    ident = const.tile([128, 128], BF16)
    make_identity(nc, ident[:])
    ident32 = const.tile([128, 128], F32)
    make_identity(nc, ident32[:])
    ones_col = const.tile([128, 1], BF16)
    nc.vector.memset(ones_col[:], 1.0)
    ones_row = const.tile([1, 128], BF16)
    nc.vector.memset(ones_row[:], 1.0)
    ones32 = const.tile([128, 1], F32)
    nc.vector.memset(ones32[:], 1.0)

    # ---------------- embedding ----------------
    ids_sb = const.tile([128, TC], I32)
    nc.sync.dma_start(ids_sb[:], t_in["ids"].ap().rearrange("(c p) -> p c", p=128))
    x = []          # token-major residual fp32, per token chunk
    for tcn in range(TC):
        xt = act.tile([128, H], F32, tag=f"xres{tcn}", name=f"xres{tcn}")
        nc.gpsimd.indirect_dma_start(
            out=xt[:], out_offset=None, in_=t_in["word_emb"].ap(),
            in_offset=bass.IndirectOffsetOnAxis(ap=ids_sb[:, tcn:tcn + 1], axis=0))
        pt = sc.tile([128, H], F32, tag="pt", name="pt")
        nc.sync.dma_start(pt[:], t_in["pos_type"].ap()[tcn * 128:(tcn + 1) * 128, :])
        nc.vector.tensor_tensor(xt[:], xt[:], pt[:], op=OP.add)
        x.append(xt)

    # LN helper: y_tiles (token-major fp32) -> x_tiles (normalised), using g/b rows
    def layernorm(y_tiles, g_bc, b_bc, out_tag):
        outs = []
        for i, y in enumerate(y_tiles):
            mu = sc.tile([128, 1], F32, tag="mu", name="mu")
            nc.vector.tensor_reduce(mu[:], y[:], axis=mybir.AxisListType.X, op=OP.add)
            nc.vector.tensor_scalar_mul(mu[:], mu[:], 1.0 / H)
            xm = sc.tile([128, H], F32, tag="xm", name="xm")
            nc.vector.tensor_scalar_sub(xm[:], y[:], mu[:, 0:1])
            sq = sc.tile([128, H], F32, tag="sq", name="sq", bufs=1)
            nvar = sc.tile([128, 1], F32, tag="nvar", name="nvar")
            nc.scalar.activation(sq[:], xm[:], AF.Square, accum_out=nvar[:])
            nc.vector.tensor_scalar_mul(nvar[:], nvar[:], 1.0 / H)
            nc.vector.tensor_scalar_add(nvar[:], nvar[:], 1e-12)
            rstd = sc.tile([128, 1], F32, tag="rstd", name="rstd")
            nc.scalar.activation(rstd[:], nvar[:], AF.Sqrt)
            nc.vector.reciprocal(rstd[:], rstd[:])
            o = act.tile([128, H], F32, tag=f"{out_tag}{i}", name=f"{out_tag}{i}")
            nc.vector.scalar_tensor_tensor(o[:], xm[:], rstd[:, 0:1], g_bc[:],
                                           op0=OP.mult, op1=OP.mult)
            nc.vector.tensor_tensor(o[:], o[:], b_bc[:], op=OP.add)
            outs.append(o)
        return outs

    def load_gb(src_ap):   # DRAM row [H] -> [128, H] broadcast
        g = lnp.tile([128, H], BF16, tag="gbc", name="gbc", bufs=2)
        nc.sync.dma_start(g[:], src_ap)
        return g

    def bcast_row(row_ap_2d):
        # row_ap_2d: DRAM AP [1, H]; broadcast over partitions
        return row_ap_2d.rearrange("one h -> one h").to_broadcast([128, H])

    embg = load_gb(t_in["emb_lngb"].ap()[0:1, :].to_broadcast([128, H]))
    embb = load_gb(t_in["emb_lngb"].ap()[1:2, :].to_broadcast([128, H]))
    x = layernorm(x, embg, embb, "xres2")

    # transpose-cast: token-major fp32 [2][128, H] -> feature-major bf16 [128, HC, 256]
    def to_featT(x_tiles, tag):
        xT = act.tile([128, HC, S], BF16, tag=tag, name=tag)
        for tcn, xt in enumerate(x_tiles):
            xb = sc.tile([128, H], BF16, tag="xcast", name="xcast")
            nc.vector.tensor_copy(xb[:], xt[:])
            nc.sync.dma_start_transpose(xT[:, :, tcn * 128:(tcn + 1) * 128], xb[:])
        return xT

    xT = to_featT(x, "xT")

    # ---------------- BERT layers ----------------
    for l in range(n_layers):
        ln1g = load_gb(t_in["lngb"].ap()[l, 0, 0:1, :].to_broadcast([128, H]))
        ln1b = load_gb(t_in["lngb"].ap()[l, 0, 1:2, :].to_broadcast([128, H]))
        ln2g = load_gb(t_in["lngb"].ap()[l, 1, 0:1, :].to_broadcast([128, H]))
        ln2b = load_gb(t_in["lngb"].ap()[l, 1, 1:2, :].to_broadcast([128, H]))

        wq = wpool.tile([128, HC, 3 * H], BF16, tag="wqkv", name="wqkv")
        nc.sync.dma_start(wq[:], t_in["wqkv"].ap()[l].rearrange("(c p) n -> p c n", p=128))
        bqk = wpool.tile([128, 2 * HC], F32, tag="bqk", name="bqk")
        nc.sync.dma_start(bqk[:], t_in["bqkT"].ap()[l])
        bvr = wpool.tile([1, H], BF16, tag="bvr", name="bvr")
        nc.sync.dma_start(bvr[:], t_in["bv_row"].ap()[l])
        wot = wpool.tile([128, HC, H], BF16, tag="wo", name="wo")
        nc.sync.dma_start(wot[:], t_in["wo"].ap()[l].rearrange("(c p) n -> p c n", p=128))
        bor = wpool.tile([1, H], BF16, tag="bor", name="bor")
        nc.sync.dma_start(bor[:], t_in["bo_row"].ap()[l])
        b1r = wpool.tile([1, FF], BF16, tag="b1r", name="b1r")
        nc.sync.dma_start(b1r[:], t_in["b1_row"].ap()[l])
        b2r = wpool.tile([1, H], BF16, tag="b2r", name="b2r")
        nc.sync.dma_start(b2r[:], t_in["b2_row"].ap()[l])

        # --- QKV (q,k feature-major; v token-major)
        qkT = act.tile([128, 2 * HC, S], BF16, tag="qkT", name="qkT")
        for mc in range(2 * HC):
            pq = ps256.tile([128, S], F32, tag="ps256", name="ps256")
            for kc in range(HC):
                nc.tensor.matmul(pq[:], wq[:, kc, mc * 128:(mc + 1) * 128],
                                 xT[:, kc, :], start=(kc == 0), stop=(kc == HC - 1))
            if mc % 2 == 0:
                nc.vector.tensor_scalar_add(qkT[:, mc, :], pq[:], bqk[:, mc:mc + 1])
            else:
                nc.scalar.activation(qkT[:, mc, :], pq[:], AF.Identity, bias=bqk[:, mc:mc + 1])
        vtm = act.tile([128, TC, H], BF16, tag="vtm", name="vtm")
        for tcn in range(TC):
            for nb in range(2):
                pv = ps384.tile([128, 384], F32, tag="ps384", name="ps384")
                for kc in range(HC):
                    nc.tensor.matmul(pv[:], xT[:, kc, tcn * 128:(tcn + 1) * 128],
                                     wq[:, kc, 2 * H + nb * 384: 2 * H + (nb + 1) * 384],
                                     start=(kc == 0), stop=False)
                nc.tensor.matmul(pv[:], ones_row[:],
                                 bvr[:, nb * 384:(nb + 1) * 384], start=False, stop=True)
                if nb % 2 == 0:
                    nc.vector.tensor_copy(vtm[:, tcn, nb * 384:(nb + 1) * 384], pv[:])
                else:
                    nc.scalar.copy(vtm[:, tcn, nb * 384:(nb + 1) * 384], pv[:])

        # --- attention: scores/exp/sums
        expT = []
        for kc2 in range(TC):
            e = act.tile([128, NH, S], BF16, tag=f"expT{kc2}", name=f"expT{kc2}")
            expT.append(e)
        s_all = act.tile([1, NH, S], F32, tag="s_all", name="s_all")
        for h in range(NH):
            base = (h % 2) * 64
            chunk = h // 2
            psum_sum = psum1.tile([1, S], F32, tag="psum1", name="psum1")
            for kc2 in range(TC):
                pss = ps256.tile([128, S], F32, tag="ps256", name="pss")
                nc.tensor.matmul(
                    pss[:],
                    qkT[base:base + 64, HC + chunk, kc2 * 128:(kc2 + 1) * 128],
                    qkT[base:base + 64, chunk, :], start=True, stop=True)
                nc.scalar.activation(expT[kc2][:, h, :], pss[:], AF.Exp, scale=0.125)
                nc.tensor.matmul(psum_sum[:], ones_col[:], expT[kc2][:, h, :],
                                 start=(kc2 == 0), stop=(kc2 == TC - 1))
            if h % 2 == 0:
                nc.vector.tensor_copy(s_all[:, h, :], psum_sum[:])
            else:
                nc.scalar.copy(s_all[:, h, :], psum_sum[:])

        # --- reciprocal path (via DRAM for the partition broadcast)
        sdram = dram.tile([NH * S], F32, tag="sdram", name="sdram")
        nc.sync.dma_start(sdram[:].rearrange("(h q) -> h q", h=NH), s_all[:, :, :])
        s_sp = sc.tile([128, NH * TC], F32, tag="s_sp", name="s_sp")
        nc.sync.dma_start(
            s_sp[:].rearrange("p (h c) -> p h c", h=NH),
            sdram[:].rearrange("(h c p) -> p h c", p=128, h=NH))
        nc.vector.reciprocal(s_sp[:], s_sp[:])
        s_bf = sc.tile([128, NH * TC], BF16, tag="s_bf", name="s_bf")
        nc.vector.tensor_copy(s_bf[:], s_sp[:])
        rdram = dram.tile([NH * S], BF16, tag="rdram", name="rdram")
        nc.sync.dma_start(
            rdram[:].rearrange("(h c p) -> p h c", p=128, h=NH),
            s_bf[:].rearrange("p (h c) -> p h c", h=NH))
        Rt = act.tile([128, NH, S], BF16, tag="Rt", name="Rt")
        nc.sync.dma_start(Rt[:], rdram[:].rearrange("(h q) -> h q", h=NH)[None, :, :].to_broadcast([128, NH, S]))

        # --- ctx (unscaled bf16) then scale
        ctxu = act.tile([128, HC, S], BF16, tag="ctxu", name="ctxu")
        for h in range(NH):
            base = (h % 2) * 64
            chunk = h // 2
            pc = ps256.tile([128, S], F32, tag="ps256", name="pc")
            for kc2 in range(TC):
                nc.tensor.matmul(pc[:64, :], vtm[:, kc2, h * 64:(h + 1) * 64],
                                 expT[kc2][:, h, :], start=(kc2 == 0), stop=(kc2 == TC - 1))
            if h % 2 == 0:
                nc.vector.tensor_copy(ctxu[base:base + 64, chunk, :], pc[:64, :])
            else:
                nc.scalar.copy(ctxu[base:base + 64, chunk, :], pc[:64, :])
        ctxT = act.tile([128, HC, S], BF16, tag="ctxT", name="ctxT")
        for chunk in range(HC):
            nc.vector.tensor_tensor(
                ctxT[0:64, chunk, :], ctxu[0:64, chunk, :], Rt[0:64, 2 * chunk, :], op=OP.mult)
            nc.vector.tensor_tensor(
                ctxT[64:128, chunk, :], ctxu[64:128, chunk, :], Rt[64:128, 2 * chunk + 1, :], op=OP.mult)

        # --- Wo + residual + LN1
        y1 = []
        for tcn in range(TC):
            yt = act.tile([128, H], F32, tag=f"y{tcn}", name=f"y1{tcn}")
            for nb in range(2):
                pao = ps384.tile([128, 384], F32, tag="ps384", name="pao")
                for dc in range(HC):
                    nc.tensor.matmul(pao[:], ctxT[:, dc, tcn * 128:(tcn + 1) * 128],
                                     wot[:, dc, nb * 384:(nb + 1) * 384],
                                     start=(dc == 0), stop=False)
                nc.tensor.matmul(pao[:], ones_row[:], bor[:, nb * 384:(nb + 1) * 384],
                                 start=False, stop=True)
                nc.vector.tensor_tensor(yt[:, nb * 384:(nb + 1) * 384], pao[:],
                                        x[tcn][:, nb * 384:(nb + 1) * 384], op=OP.add)
            y1.append(yt)
        x = layernorm(y1, ln1g, ln1b, "xres")
        xT2 = to_featT(x, "xT2")

        # --- FFN (token-major hdn blocks, DMA-transposed to feature-major)
        hdnT = act.tile([128, FC, S], BF16, tag="hdnT", name="hdnT")
        for nb in range(FF // 384):
            w1cs = []
            for kc in range(HC):
                w1c = w2pool.tile([128, 384], BF16, tag="w1c", name="w1c", bufs=7)
                nc.sync.dma_start(w1c[:], t_in["w1"].ap()[l, kc * 128:(kc + 1) * 128,
                                                          nb * 384:(nb + 1) * 384])
                w1cs.append(w1c)
            for tcn in range(TC):
                ph = ps384.tile([128, 384], F32, tag="ps384", name="ph")
                for kc in range(HC):
                    nc.tensor.matmul(ph[:], xT2[:, kc, tcn * 128:(tcn + 1) * 128],
                                     w1cs[kc][:], start=(kc == 0), stop=False)
                nc.tensor.matmul(ph[:], ones_row[:], b1r[:, nb * 384:(nb + 1) * 384],
                                 start=False, stop=True)
                hdn_c = sc.tile([128, 384], BF16, tag="hdnc", name="hdnc", bufs=3)
                nc.scalar.activation(hdn_c[:], ph[:], AF.Gelu)
                nc.sync.dma_start_transpose(
                    hdnT[:, nb * 3:(nb + 1) * 3, tcn * 128:(tcn + 1) * 128], hdn_c[:])
        pfs = [[ps384.tile([128, 384], F32, tag="ps384", name=f"pf{t}{n}")
                for n in range(2)] for t in range(TC)]
        for fc in range(FC):
            w2c = w2pool.tile([128, H], BF16, tag="w2c", name="w2c")
            nc.sync.dma_start(w2c[:], t_in["w2"].ap()[l, fc * 128:(fc + 1) * 128, :])
            for tcn in range(TC):
                for nb in range(2):
                    nc.tensor.matmul(pfs[tcn][nb][:], hdnT[:, fc, tcn * 128:(tcn + 1) * 128],
                                     w2c[:, nb * 384:(nb + 1) * 384],
                                     start=(fc == 0), stop=False)
        y2 = []
        for tcn in range(TC):
            yt = act.tile([128, H], F32, tag=f"y{tcn}", name=f"y2{tcn}")
            for nb in range(2):
                nc.tensor.matmul(pfs[tcn][nb][:], ones_row[:], b2r[:, nb * 384:(nb + 1) * 384],
                                 start=False, stop=True)
                nc.vector.tensor_tensor(yt[:, nb * 384:(nb + 1) * 384], pfs[tcn][nb][:],
                                        x[tcn][:, nb * 384:(nb + 1) * 384], op=OP.add)
            y2.append(yt)
        x = layernorm(y2, ln2g, ln2b, "xres2")
        xT = to_featT(x, "xT")

    if dbg:
        for tcn in range(TC):
            nc.sync.dma_start(dbg["x_out"].ap()[tcn * 128:(tcn + 1) * 128, :], x[tcn][:])

    # free BERT-only psum pools before the LSTM phase (LIFO order)
    for pname in ("ps384", "psum1"):
        ctx_pools.pop(pname).__exit__(None, None, None)


    # ---------------- BiLSTM ----------------
    # xg precompute helper: out_sb [128, 4, S] fp32 from contraction tiles
    def xg_precompute(w_ap, nchunks, rhs_fn, bias_tile, tag):
        xg = act.tile([128, 4, S], BF16, tag="xg" + tag[-1], name=tag)
        wt = wpool.tile([128, nchunks, 4 * R], BF16, tag=f"w{tag}", name=f"w{tag}")
        nc.sync.dma_start(wt[:], w_ap.rearrange("(c p) n -> p c n", p=128))
        for gc in range(4):
            pg = ps256.tile([128, S], F32, tag="ps256", name="pg")
            for kc in range(nchunks):
                nc.tensor.matmul(pg[:], wt[:, kc, gc * 128:(gc + 1) * 128], rhs_fn(kc),
                                 start=(kc == 0), stop=(kc == nchunks - 1))
            nc.scalar.activation(xg[:, gc, :], pg[:], AF.Identity, bias=bias_tile[:, gc:gc + 1])
        return xg

    xb1 = const.tile([128, 4, 2], F32)
    nc.sync.dma_start(xb1[:], t_in["xb1T"].ap().rearrange("d p g -> p g d"))
    xb2 = const.tile([128, 4, 2], F32)
    nc.sync.dma_start(xb2[:], t_in["xb2T"].ap().rearrange("d p g -> p g d"))

    whh1t = const.tile([128, 2, 4 * R], BF16)
    nc.sync.dma_start(whh1t[:], t_in["whh1"].ap().rearrange("d p n -> p d n"))
    whh2t = const.tile([128, 2, 4 * R], BF16)
    nc.sync.dma_start(whh2t[:], t_in["whh2"].ap().rearrange("d p n -> p d n"))

    def lstm_layer(xg_f, xg_b, whht, hs_tag):
        # preload XG psum (2 banks per dir), run both chains interleaved
        XGs = []
        hss = []
        xg_pools = []
        for d, xg in ((0, xg_f), (1, xg_b)):
            XGcm = tc.tile_pool(name=f"XG{hs_tag}{d}", bufs=1, space="PSUM")
            XG = XGcm.__enter__()
            xg_pools.append(XGcm)
            X = XG.tile([128, S, 4], F32, tag=f"XG{d}", name=f"XG{d}")
            for b in range(S * 4 // 512):
                nc.tensor.matmul(X[:, b * 128:(b + 1) * 128, :], ident[:],
                                 xg[:, :, b * 128:(b + 1) * 128].transpose([0, 2, 1]),
                                 start=True, stop=False, skip_group_check=True)
            XGs.append(X)
            hs = act.tile([128, S + 1], BF16, tag=f"hs{hs_tag}{d}", name=f"hs{hs_tag}{d}")
            nc.vector.memset(hs[:, 0:1], 0.0)
            hss.append(hs)
        cs = [sc.tile([128, 1], F32, tag=f"c{d}", name=f"c{d}") for d in range(2)]
        for d in range(2):
            nc.vector.memset(cs[d][:], 0.0)
        for t in range(S):
            for d in range(2):
                X, hs, c = XGs[d], hss[d], cs[d]
                for j in range(4):
                    nc.tensor.matmul(X[:, t, j:j + 1], whht[:, d, j * R:(j + 1) * R],
                                     hs[:, t:t + 1], start=False,
                                     stop=(t == S - 1 and j == 3), skip_group_check=True)
                sig = sc.tile([128, 4], F32, tag=f"sig{d}", name=f"sig{d}", bufs=3)
                nc.scalar.activation(sig[:], X[:, t, :], AF.Sigmoid)
                nc.vector._custom_dve(LSTM_C_UPDATE, out=c[:], in0=c[:], in1=sig[:, 2:3],
                                      s0=sig[:, 1:2], s1=sig[:, 0:1])
                tch = sc.tile([128, 1], F32, tag=f"tch{d}", name=f"tch{d}", bufs=3)
                nc.scalar.activation(tch[:], c[:], AF.Tanh)
                nc.vector.tensor_tensor(hs[:, t + 1:t + 2], sig[:, 3:4], tch[:], op=OP.mult)
        for XGp in reversed(xg_pools):
            XGp.__exit__(None, None, None)
        return hss

    # layer 1: forward dir reads xT natural, backward reads xT time-reversed
    def rev(ap):   # reverse the last (time) axis of [128, S] AP
        return ap[:, ::-1]

    xg1f = xg_precompute(t_in["wih1"].ap()[0], HC, lambda kc: xT[:, kc, :], xb1[:, :, 0], "xg1f")
    xg1b = xg_precompute(t_in["wih1"].ap()[1], HC, lambda kc: rev(xT[:, kc, :]), xb1[:, :, 1], "xg1b")
    hs1 = lstm_layer(xg1f, xg1b, whh1t, "1")

    # layer 2: input = [f1; b1]; forward: f natural + b reversed; backward: f reversed + b natural
    xg2f = xg_precompute(
        t_in["wih2"].ap()[0], 2,
        lambda kc: hs1[0][:, 1:S + 1] if kc == 0 else rev(hs1[1][:, 1:S + 1]),
        xb2[:, :, 0], "xg2f")
    xg2b = xg_precompute(
        t_in["wih2"].ap()[1], 2,
        lambda kc: rev(hs1[0][:, 1:S + 1]) if kc == 0 else hs1[1][:, 1:S + 1],
        xb2[:, :, 1], "xg2b")
    hs2 = lstm_layer(xg2f, xg2b, whh2t, "2")

    if dbg:
        for i, hsx in enumerate(hs1 + hs2):
            h32 = sc.tile([128, S], F32, tag="h32", name="h32")
            nc.vector.tensor_copy(h32[:], hsx[:, 1:S + 1])
            nc.sync.dma_start(dbg["hs_out"].ap()[i], h32[:])

    # ---------------- classifier ----------------
    clsw = const.tile([128, 2, T], BF16)
    nc.sync.dma_start(clsw[:], t_in["clsW"].ap().rearrange("(c p) t -> p c t", p=128))
    clsb_sb = const.tile([T, 1], F32)
    nc.sync.dma_start(clsb_sb[:], t_in["clsb"].ap())
    pem = ps256.tile([T, S], F32, tag="ps256", name="pem")
    nc.tensor.matmul(pem[:], clsw[:, 0, :], hs2[0][:, 1:S + 1], start=True, stop=False)
    nc.tensor.matmul(pem[:], clsw[:, 1, :], rev(hs2[1][:, 1:S + 1]), start=False, stop=True)
    emT = const.tile([T, S], F32)
    nc.scalar.activation(emT[:], pem[:], AF.Identity, bias=clsb_sb[:, 0:1])
    if dbg:
        nc.sync.dma_start(dbg["em_out"].ap(), emT[:])

    # ---------------- CRF ----------------
    trans_sb = const.tile([T, T], F32)
    nc.sync.dma_start(trans_sb[:], t_in["crf_trans"].ap())
    start_sb = const.tile([T, 1], F32)
    nc.sync.dma_start(start_sb[:], t_in["crf_start"].ap())
    end_sb = const.tile([T, 1], F32)
    nc.sync.dma_start(end_sb[:], t_in["crf_end"].ap())
    expM = const.tile([T, T], F32)
    nc.scalar.activation(expM[:], trans_sb[:], AF.Exp)
    expEm = const.tile([T, S], F32)
    nc.scalar.activation(expEm[:], emT[:], AF.Exp)

    logs = const.tile([1, 64], F32)   # renorm log collector
    nc.vector.memset(logs[:], 0.0)
    n_logs = [0]
    psc = pool("psc", 2, space="PSUM")

    def log_and_renorm(vec, width, tag):
        # vec [T, width] sbuf; compute total sum -> logs[n], scale vec by 1/sum
        pssum = psc.tile([1, T], F32, tag="cs", name="cs", bufs=1)
        nc.tensor.matmul(pssum[:, :width] if width < T else pssum[:],
                         ones32[:T, :], vec[:], start=True, stop=True)
        tot = sc.tile([1, 1], F32, tag="tot", name="tot")
        nc.vector.tensor_reduce(tot[:], pssum[:, :width] if width < T else pssum[:],
                                axis=mybir.AxisListType.X, op=OP.add)
        nc.scalar.activation(logs[:, n_logs[0]:n_logs[0] + 1], tot[:], AF.Ln)
        n_logs[0] += 1
        rec = sc.tile([1, 1], F32, tag="rec", name="rec")
        nc.vector.reciprocal(rec[:], tot[:])
        recb = sc.tile([T, 1], F32, tag="recb", name="recb")
        nc.gpsimd.partition_broadcast(recb[:], rec[:])
        nc.vector.tensor_scalar_mul(vec[:], vec[:], recb[:, 0:1])

    HALF = S // 2
    # chain A: probability vector scan over t = 1..HALF-1 (p0 at t=0)
    p_vec = const.tile([T, 1], F32)
    nc.scalar.activation(p_vec[:], emT[:, 0:1], AF.Exp, bias=start_sb[:, 0:1])
    for t in range(1, HALF):
        pp = psc.tile([T, 1], F32, tag="pp", name="pp")
        nc.tensor.matmul(pp[:], expM[:], p_vec[:], start=True, stop=True)
        nc.vector.tensor_tensor(p_vec[:], pp[:], expEm[:, t:t + 1], op=OP.mult)
        if t % RENORM == 0:
            log_and_renorm(p_vec, 1, "pA")

    # chain B: S_t = Mtilde_t^T . S_{t-1}, t = HALF..S-1 ; Mtilde precomputed
    emB = const.tile([T, S - HALF, T], F32)   # emB[k, t, j] = expEm[j, HALF+t] (bcast over k)
    emdram = dram.tile([T * (S - HALF)], F32, tag="emd", name="emd")
    nc.sync.dma_start(emdram[:].rearrange("(t j) -> j t", j=T), expEm[:, HALF:S])
    nc.sync.dma_start(emB[:], emdram[:].rearrange("(t j) -> t j", j=T)[None, :, :].to_broadcast([T, S - HALF, T]))
    Mt = const.tile([T, S - HALF, T], F32)
    nc.vector.tensor_tensor(Mt[:], emB[:], expM[:][:, None, :].to_broadcast([T, S - HALF, T]), op=OP.mult)
    S_mat = const.tile([T, T], F32)
    nc.vector.tensor_copy(S_mat[:], ident32[:T, :T])
    for t in range(S - HALF):
        ps_ = psc.tile([T, T], F32, tag="pp", name="ppS")
        nc.tensor.matmul(ps_[:], Mt[:, t, :], S_mat[:], start=True, stop=True)
        if (t + 1) % RENORM == 0:
            nc.vector.tensor_copy(S_mat[:], ps_[:])
            log_and_renorm(S_mat, T, "SB")
        else:
            nc.vector.tensor_copy(S_mat[:], ps_[:])

    # combine: p_final = S_final^T @ p_mid ; denom = ln(sum_j p_final * exp(end)) + sum(logs)
    pSt = psc.tile([T, T], F32, tag="pp", name="pSt")
    nc.tensor.transpose(pSt[:], S_mat[:], ident32[:T, :T])
    St_T = const.tile([T, T], F32)
    nc.vector.tensor_copy(St_T[:], pSt[:])
    pfin = psc.tile([T, 1], F32, tag="pp", name="pfin")
    nc.tensor.matmul(pfin[:], St_T[:], p_vec[:], start=True, stop=True)
    expEnd = const.tile([T, 1], F32)
    nc.scalar.activation(expEnd[:], end_sb[:], AF.Exp)
    pfe = const.tile([T, 1], F32)
    nc.vector.tensor_tensor(pfe[:], pfin[:], expEnd[:], op=OP.mult)
    pden = psc.tile([1, T], F32, tag="cs", name="pden", bufs=1)
    nc.tensor.matmul(pden[:, 0:1], ones32[:T, :], pfe[:], start=True, stop=True)
    denom = const.tile([1, 1], F32)
    nc.scalar.activation(denom[:], pden[:, 0:1], AF.Ln)
    logsum = const.tile([1, 1], F32)
    nc.vector.tensor_reduce(logsum[:], logs[:], axis=mybir.AxisListType.X, op=OP.add)
    nc.vector.tensor_tensor(denom[:], denom[:], logsum[:], op=OP.add)

    # ---------------- numerator ----------------
    tags_b = const.tile([T, S], F32)
    nc.sync.dma_start(tags_b[:], t_in["tags_f"].ap()[None, :].to_broadcast([T, S]))
    iota_c = const.tile([T, 1], I32)
    nc.gpsimd.iota(iota_c[:], pattern=[[0, 1]], base=0, channel_multiplier=1)
    iota_f = const.tile([T, 1], F32)
    nc.vector.tensor_copy(iota_f[:], iota_c[:])
    onehot = const.tile([T, S], F32)
    nc.vector.tensor_scalar(onehot[:], tags_b[:], iota_f[:, 0:1], None,
                            op0=OP.is_equal)
    # em-gold: sum over (t,s) of emT*onehot ; start/end-gold via onehot cols
    emoh = const.tile([T, S], F32)
    gold1 = const.tile([T, 1], F32)
    nc.vector.tensor_tensor(emoh[:], emT[:], onehot[:], op=OP.mult)
    nc.vector.tensor_reduce(gold1[:], emoh[:], axis=mybir.AxisListType.X, op=OP.add)
    seg = const.tile([T, 1], F32)
    nc.vector.tensor_tensor(seg[:], start_sb[:], onehot[:, 0:1], op=OP.mult)
    nc.vector.tensor_tensor(gold1[:], gold1[:], seg[:], op=OP.add)
    nc.vector.tensor_tensor(seg[:], end_sb[:], onehot[:, S - 1:S], op=OP.mult)
    nc.vector.tensor_tensor(gold1[:], gold1[:], seg[:], op=OP.add)
    # trans-gold: A = trans^T-sel: A[j, s] = trans[tag_s, j] = sum_i trans[i,j]*onehot[i,s]
    pA = psc.tile([T, S], F32, tag="pAo", name="pA", bufs=1)
    nc.tensor.matmul(pA[:, 0:S - 1], trans_sb[:], onehot[:, 0:S - 1], start=True, stop=True)
    tg = const.tile([T, S], F32)
    nc.vector.tensor_tensor(tg[:, 0:S - 1], pA[:, 0:S - 1], onehot[:, 1:S], op=OP.mult)
    tgs = const.tile([T, 1], F32)
    nc.vector.tensor_reduce(tgs[:], tg[:, 0:S - 1], axis=mybir.AxisListType.X, op=OP.add)
    nc.vector.tensor_tensor(gold1[:], gold1[:], tgs[:], op=OP.add)
    pnum = psc.tile([1, T], F32, tag="cs", name="pnum", bufs=1)
    nc.tensor.matmul(pnum[:, 0:1], ones32[:T, :], gold1[:], start=True, stop=True)

    # partial = 0.5 * (denom - num)
    part = const.tile([1, 1], F32)
    nc.vector.tensor_tensor(part[:], denom[:], pnum[:, 0:1], op=OP.subtract)
    nc.vector.tensor_scalar_mul(part[:], part[:], 0.5)
    if dbg:
        dn = const.tile([1, 4], F32)
        nc.vector.tensor_copy(dn[:, 0:1], denom[:])
        nc.vector.tensor_copy(dn[:, 1:2], pnum[:, 0:1])
        nc.vector.tensor_copy(dn[:, 2:3], logsum[:])
        nc.vector.tensor_copy(dn[:, 3:4], part[:])
        nc.sync.dma_start(dbg["dn_out"].ap(), dn[:])

    # ---------------- final AllReduce ----------------
    bin_ = dram.tile([1, 1], F32, tag="arin", name="arin")
    bout = dram.tile([1, 1], F32, tag="arout", name="arout")
    nc.sync.dma_start(bin_[:], part[:])
    nc.gpsimd.collective_compute(
        "AllReduce", OP.add, replica_groups=[list(range(N_CORES))],
        ins=[bin_[:].opt()], outs=[bout[:].opt()])
    nc.sync.dma_start(t_out.ap(), bout[:])

    for p in reversed(list(ctx_pools.values())):
        p.__exit__(None, None, None)


# ---------------------------------------------------------------- host prep

def _bf16(a):
    return np.asarray(a, np.float32).astype(ml_dtypes.bfloat16)


def prepare_inputs(input_ids, attention_mask, tags, params, n_layers=L):
    p = params
    lay = p["layers"]
    per_core = []

    wqkv = np.concatenate([lay["Wq"], lay["Wk"], lay["Wv"]], axis=2)  # [L, H, 3H]
    bqk = np.concatenate([lay["bq"][:n_layers], lay["bk"][:n_layers]], axis=1)
    bqkT = bqk.reshape(n_layers, 2 * HC, 128).transpose(0, 2, 1)      # [L, 128, 12]
    lngb = np.stack([
        np.stack([lay["ln1_g"], lay["ln1_b"]], axis=1),
        np.stack([lay["ln2_g"], lay["ln2_b"]], axis=1)], axis=1)      # [L, 2, 2, H]

    def lstm_dir(lp, sfx):
        wih = np.asarray(lp[f"Wih_{sfx}"], np.float32).T.copy()   # [in, 4R]
        whh = np.asarray(lp[f"Whh_{sfx}"], np.float32).T.copy()   # [R, 4R]
        xb = np.asarray(lp[f"bih_{sfx}"], np.float32) + np.asarray(lp[f"bhh_{sfx}"], np.float32)
        wih[:, 2 * R:3 * R] *= 2.0
        whh[:, 2 * R:3 * R] *= 2.0
        xb = xb.copy()
        xb[2 * R:3 * R] *= 2.0
        return wih, whh, xb.reshape(4, R).T    # xbT [128, 4]

    w1f, h1f, b1f = lstm_dir(p["lstm0"], "f")
    w1b, h1b, b1b = lstm_dir(p["lstm0"], "b")
    w2f, h2f, b2f = lstm_dir(p["lstm1"], "f")
    w2b, h2b, b2b = lstm_dir(p["lstm1"], "b")

    shared = {
        "word_emb": np.asarray(p["word_emb"], np.float32),
        "pos_type": np.asarray(p["pos_emb"][:S], np.float32) + np.asarray(p["type_emb"][0], np.float32)[None, :],
        "emb_lngb": _bf16(np.stack([p["emb_ln_g"], p["emb_ln_b"]])),
        "wqkv": _bf16(wqkv[:n_layers]),
        "bqkT": np.ascontiguousarray(bqkT[:n_layers], dtype=np.float32),
        "bv_row": _bf16(lay["bv"][:n_layers, None, :]),
        "wo": _bf16(lay["Wo"][:n_layers]),
        "bo_row": _bf16(lay["bo"][:n_layers, None, :]),
        "w1": _bf16(lay["W1"][:n_layers]),
        "b1_row": _bf16(lay["b1"][:n_layers, None, :]),
        "w2": _bf16(lay["W2"][:n_layers]),
        "b2_row": _bf16(lay["b2"][:n_layers, None, :]),
        "lngb": _bf16(lngb[:n_layers]),
        "wih1": np.stack([_bf16(w1f), _bf16(w1b)]),
        "whh1": np.stack([_bf16(h1f), _bf16(h1b)]),
        "xb1T": np.stack([b1f, b1b]).astype(np.float32),
        "wih2": np.stack([_bf16(w2f), _bf16(w2b)]),
        "whh2": np.stack([_bf16(h2f), _bf16(h2b)]),
        "xb2T": np.stack([b2f, b2b]).astype(np.float32),
        "clsW": _bf16(p["cls_W"]),
        "clsb": np.asarray(p["cls_b"], np.float32)[:, None],
        "crf_trans": np.asarray(p["crf_trans"], np.float32),
        "crf_start": np.asarray(p["crf_start"], np.float32)[:, None],
        "crf_end": np.asarray(p["crf_end"], np.float32)[:, None],
    }
    ids = np.asarray(input_ids, np.int64).astype(np.int32)
    tg = np.asarray(tags, np.int64).astype(np.int32)
    for c in range(N_CORES):
        b = c // 2
        m = dict(shared)
        m["ids"] = np.ascontiguousarray(ids[b])
        m["tags"] = np.ascontiguousarray(tg[b])
        m["tags_f"] = np.ascontiguousarray(tg[b].astype(np.float32))
        per_core.append(m)
    return per_core


_CACHE = {}


def _get_program(n_layers=L, debug=False):
    key = (n_layers, debug)
    if key not in _CACHE:
        _CACHE[key] = build_program(n_layers, debug)
    return _CACHE[key]


def kernel(input_ids, attention_mask, tags, params, n_layers=L, debug=False, trace=False):
    nc = _get_program(n_layers, debug)
    in_maps = prepare_inputs(input_ids, attention_mask, tags, params, n_layers)
    try:
        res = run_bass_kernel_spmd(nc, in_maps, list(range(N_CORES)), trace=trace)
    except Exception:
        # transient NRT_EXEC_UNIT_UNRECOVERABLE wedges clear on retry
        res = run_bass_kernel_spmd(nc, in_maps, list(range(N_CORES)), trace=trace)
    out = np.float32(res.results[0]["out"][0, 0])
    if debug or trace:
        kernel.last_results = res
    return np.asarray(out, dtype=np.float32).reshape(())
